# revision 1
# baseline (speedup 1.0000x reference)
"""Sparse 3D conv (rulebook gather -> GEMM -> accumulate) on 8 TRN2 NeuronCores.

Strategy (data-parallel over output sites, no collectives):
  - Replicate the feats table (bf16, with a trailing zero row for invalid
    rulebook entries) and the [27,64,64] kernel on every core.
  - Shard the 400k output sites 50k/core; each core gathers its neighbor
    rows via SWDGE indirect DMA (one big gather per 512-site tile covering
    all 27 kernel offsets), transposes gathered [site, cin] tiles to
    [cin, site] on the TensorEngine (pairs of offsets packed to K=128),
    and accumulates 14 K=128-packed bf16 matmuls into f32 PSUM.
  - Bias add fused into the PSUM->SBUF copy, output transposed back to
    site-major on the TensorEngine and DMAd out contiguously.
"""

import os
import sys
from contextlib import ExitStack

sys.path.insert(0, "/opt/trn_rl_repo")

import ml_dtypes
import numpy as np

import concourse.bass as bass
import concourse.tile as tile
from concourse import bacc, mybir
from concourse.bass_utils import run_bass_kernel_spmd
from concourse.masks import make_identity

BF16 = ml_dtypes.bfloat16

# Problem constants (hardcoded per contract)
N = 400000
CIN = 64
COUT = 64
KVOL = 27
NCORES = 8
SPC = N // NCORES  # sites per core = 50000

TILE = 512  # sites per device tile
NPAIRS = (KVOL + 1) // 2  # 14 (27 offsets + 1 zero pad)
IDX_PER_TILE = NPAIRS * 8  # 112 indices per partition per tile


class Cfg:
    def __init__(self, n_rows, n_tiles):
        self.n_rows = n_rows  # feats table rows incl. zero pad rows
        self.n_tiles = n_tiles  # site tiles per core
        self.spad = n_tiles * TILE  # padded sites per core


N_SWDGE_QUEUES = 1  # spread indirect gathers across SWDGE queues when >1


def build(cfg: Cfg):
    """Build + compile the per-core Bass program. Returns (nc, names)."""
    nc = bacc.Bacc(
        "TRN2",
        target_bir_lowering=False,
        debug=False,
        num_devices=NCORES,
        num_swdge_queues=N_SWDGE_QUEUES,
    )
    f32 = mybir.dt.float32
    bf16 = mybir.dt.bfloat16
    i32 = mybir.dt.int32

    tbl = nc.dram_tensor("tbl", [cfg.n_rows, CIN], bf16, kind="ExternalInput")
    idxs = nc.dram_tensor(
        "idxs", [cfg.n_tiles, 128, IDX_PER_TILE], i32, kind="ExternalInput"
    )
    wts = nc.dram_tensor("wts", [128, NPAIRS * COUT], bf16, kind="ExternalInput")
    biasd = nc.dram_tensor("bias", [COUT, 1], f32, kind="ExternalInput")
    outd = nc.dram_tensor("out", [cfg.spad, COUT], f32, kind="ExternalOutput")

    with tile.TileContext(nc) as tc, ExitStack() as ctx:
        const = ctx.enter_context(tc.tile_pool(name="const", bufs=1))
        idf = const.tile([128, 128], f32)
        make_identity(nc, idf[:])
        idb = const.tile([128, 128], bf16)
        nc.vector.tensor_copy(idb[:], idf[:])
        wt = const.tile([128, NPAIRS * COUT], bf16)
        nc.sync.dma_start(wt[:], wts[:])
        bt = const.tile([COUT, 1], f32)
        nc.sync.dma_start(bt[:], biasd[:])

        ip = ctx.enter_context(tc.tile_pool(name="ip", bufs=3))
        gp = ctx.enter_context(tc.tile_pool(name="gp", bufs=6))
        tp = ctx.enter_context(tc.tile_pool(name="tp", bufs=2, space="PSUM"))
        rp = ctx.enter_context(tc.tile_pool(name="rp", bufs=6))
        app = ctx.enter_context(tc.tile_pool(name="ap", bufs=2, space="PSUM"))
        bp = ctx.enter_context(tc.tile_pool(name="bp", bufs=2))
        otp = ctx.enter_context(tc.tile_pool(name="otp", bufs=2, space="PSUM"))
        osp = ctx.enter_context(tc.tile_pool(name="osp", bufs=3))

        gq = [0]

        def gather(out_ap, idx_ap):
            inst = nc.gpsimd.indirect_dma_start(
                out=out_ap,
                out_offset=None,
                in_=tbl[:],
                in_offset=bass.IndirectOffsetOnAxis(ap=idx_ap, axis=0),
            )
            if N_SWDGE_QUEUES > 1:
                q = gq[0] % N_SWDGE_QUEUES
                gq[0] += 1
                if q:
                    inst.ins.queue = f"qPoolDynamic{q}"
            return inst

        for t in range(cfg.n_tiles):
            it = ip.tile([128, IDX_PER_TILE], i32)
            nc.sync.dma_start(it[:], idxs[t])
            acc = app.tile([COUT, TILE], f32)
            for pr in range(NPAIRS):
                # HW indirect DMA: one index per partition per instruction.
                # 8 gathers fill g [128, 512]: chunk c, lane w at cols
                # c*128 + w*64. Index column q = pr*8 + c*2 + w.
                g = gp.tile([128, 512], bf16)
                for c in range(4):
                    for w in range(2):
                        q = pr * 8 + c * 2 + w
                        gather(
                            g[:, c * 128 + w * 64 : c * 128 + (w + 1) * 64],
                            it[:, q : q + 1],
                        )
                tpt = tp.tile([128, TILE], bf16)
                for c in range(4):
                    nc.tensor.transpose(
                        out=tpt[:, c * 128 : (c + 1) * 128],
                        in_=g[:, c * 128 : (c + 1) * 128],
                        identity=idb[:],
                    )
                r = rp.tile([128, TILE], bf16)
                if pr % 2 == 0:
                    nc.vector.tensor_copy(r[:], tpt[:])
                else:
                    nc.scalar.copy(r[:], tpt[:])
                nc.tensor.matmul(
                    acc[:],
                    wt[:, pr * COUT : (pr + 1) * COUT],
                    r[:],
                    start=(pr == 0),
                    stop=(pr == NPAIRS - 1),
                )
            ob = bp.tile([COUT, TILE], f32)
            nc.vector.tensor_add(
                out=ob[:], in0=acc[:], in1=bt[:].to_broadcast([COUT, TILE])
            )
            ot = otp.tile([128, 4 * COUT], f32)
            for c in range(4):
                nc.tensor.transpose(
                    out=ot[:, c * COUT : (c + 1) * COUT],
                    in_=ob[:, c * 128 : (c + 1) * 128],
                    identity=idf[:COUT, :COUT],
                )
            os_ = osp.tile([128, 4 * COUT], f32)
            nc.scalar.copy(os_[:], ot[:])
            nc.sync.dma_start(
                outd[t * TILE : (t + 1) * TILE, :].rearrange(
                    "(c p) ci -> p c ci", p=128
                ),
                os_[:].rearrange("p (c ci) -> p c ci", c=4),
            )

    nc.compile()
    return nc


def prep_inputs(feats, kern, bias, neighbor_map, cfg: Cfg, n_sites_total, n_cores):
    """Host-side marshalling into per-core input maps."""
    zrow = n_sites_total  # index of the zero row in the padded table
    assert cfg.n_rows > zrow

    tblh = np.zeros((cfg.n_rows, CIN), dtype=BF16)
    tblh[: feats.shape[0]] = feats.astype(BF16)

    nm = np.asarray(neighbor_map)
    idx32 = np.where(nm >= 0, nm, zrow).astype(np.int32)  # [KVOL, n_sites]

    w_pk = np.zeros((NPAIRS, 128, COUT), dtype=np.float32)
    for pr in range(NPAIRS):
        k0, k1 = 2 * pr, 2 * pr + 1
        w_pk[pr, :CIN] = kern[k0]
        if k1 < KVOL:
            w_pk[pr, CIN:] = kern[k1]
    wtsh = np.ascontiguousarray(
        w_pk.transpose(1, 0, 2).reshape(128, NPAIRS * COUT)
    ).astype(BF16)

    biash = np.ascontiguousarray(bias.reshape(COUT, 1)).astype(np.float32)

    spc = n_sites_total // n_cores
    in_maps = []
    for c in range(n_cores):
        sl = idx32[:, c * spc : (c + 1) * spc]  # [27, spc]
        padn = cfg.spad - spc
        a = np.concatenate(
            [
                np.concatenate(
                    [sl, np.full((KVOL, padn), zrow, np.int32)], axis=1
                ),
                np.full((1, cfg.spad), zrow, np.int32),
            ],
            axis=0,
        )  # [28, spad]
        a = a.reshape(2 * NPAIRS, cfg.n_tiles, 4, 128)  # [k, t, c, p]
        a = a.reshape(NPAIRS, 2, cfg.n_tiles, 4, 128).transpose(2, 4, 0, 3, 1)
        idxh = np.ascontiguousarray(
            a.reshape(cfg.n_tiles, 128, IDX_PER_TILE)
        )
        in_maps.append({"tbl": tblh, "idxs": idxh, "wts": wtsh, "bias": biash})
    return in_maps


_CACHE = {}


def kernel(feats, kernel, bias, neighbor_map):
    feats = np.asarray(feats, dtype=np.float32)
    kern = np.asarray(kernel, dtype=np.float32)
    bias = np.asarray(bias, dtype=np.float32)

    n_tiles = (SPC + TILE - 1) // TILE  # 98
    cfg = Cfg(n_rows=N + 128, n_tiles=n_tiles)

    if "nc" not in _CACHE:
        _CACHE["nc"] = build(cfg)
    nc = _CACHE["nc"]

    in_maps = prep_inputs(feats, kern, bias, neighbor_map, cfg, N, NCORES)
    res = run_bass_kernel_spmd(nc, in_maps, list(range(NCORES)))
    out = np.concatenate(
        [res.results[i]["out"][:SPC] for i in range(NCORES)], axis=0
    )
    return out.astype(np.float32)


if __name__ == "__main__":
    # smoke test with random data
    rng = np.random.default_rng(0)
    feats = rng.standard_normal((N, CIN), dtype=np.float32)
    kern = rng.standard_normal((KVOL, CIN, COUT), dtype=np.float32) * 0.02
    bias = rng.standard_normal(COUT).astype(np.float32) * 0.02
    nm = rng.integers(0, N, (KVOL, N))
    out = kernel(feats, kern, bias, nm)
    print(out.shape, out.dtype)



# revision 2
# speedup vs baseline: 1.4029x; 1.4029x over previous
"""Sparse 3D conv (rulebook gather -> GEMM -> accumulate) on 8 TRN2 NeuronCores.

Strategy (data-parallel over output sites, no collectives):
  - Replicate the feats table (bf16, with a trailing zero row for invalid
    rulebook entries) and the [27,64,64] kernel on every core.
  - Shard the 400k output sites 50k/core; each core gathers its neighbor
    rows via SWDGE indirect DMA (one big gather per 512-site tile covering
    all 27 kernel offsets), transposes gathered [site, cin] tiles to
    [cin, site] on the TensorEngine (pairs of offsets packed to K=128),
    and accumulates 14 K=128-packed bf16 matmuls into f32 PSUM.
  - Bias add fused into the PSUM->SBUF copy, output transposed back to
    site-major on the TensorEngine and DMAd out contiguously.
"""

import os
import sys
from contextlib import ExitStack

sys.path.insert(0, "/opt/trn_rl_repo")

import ml_dtypes
import numpy as np

import concourse.bass as bass
import concourse.tile as tile
from concourse import bacc, mybir
from concourse.bass_utils import run_bass_kernel_spmd
from concourse.masks import make_identity

BF16 = ml_dtypes.bfloat16

# Problem constants (hardcoded per contract)
N = 400000
CIN = 64
COUT = 64
KVOL = 27
NCORES = 8
SPC = N // NCORES  # sites per core = 50000

TILE = 512  # sites per device tile
NPAIRS = (KVOL + 1) // 2  # 14 (27 offsets + 1 zero pad)
IDX_PER_TILE = NPAIRS * 8  # 112 indices per partition per tile


class Cfg:
    def __init__(self, n_rows, n_tiles):
        self.n_rows = n_rows  # feats table rows incl. zero pad rows
        self.n_tiles = n_tiles  # site tiles per core
        self.spad = n_tiles * TILE  # padded sites per core


N_SWDGE_QUEUES = 4  # spread indirect gathers across SWDGE queues when >1


def build(cfg: Cfg):
    """Build + compile the per-core Bass program. Returns (nc, names)."""
    nc = bacc.Bacc(
        "TRN2",
        target_bir_lowering=False,
        debug=False,
        num_devices=NCORES,
        num_swdge_queues=N_SWDGE_QUEUES,
    )
    f32 = mybir.dt.float32
    bf16 = mybir.dt.bfloat16
    i32 = mybir.dt.int32

    tbl = nc.dram_tensor("tbl", [cfg.n_rows, CIN], bf16, kind="ExternalInput")
    idxs = nc.dram_tensor(
        "idxs", [cfg.n_tiles, 128, IDX_PER_TILE], i32, kind="ExternalInput"
    )
    wts = nc.dram_tensor("wts", [128, NPAIRS * COUT], bf16, kind="ExternalInput")
    biasd = nc.dram_tensor("bias", [COUT, 1], f32, kind="ExternalInput")
    outd = nc.dram_tensor("out", [cfg.spad, COUT], f32, kind="ExternalOutput")

    with tile.TileContext(nc) as tc, ExitStack() as ctx:
        const = ctx.enter_context(tc.tile_pool(name="const", bufs=1))
        idf = const.tile([128, 128], f32)
        make_identity(nc, idf[:])
        idb = const.tile([128, 128], bf16)
        nc.vector.tensor_copy(idb[:], idf[:])
        wt = const.tile([128, NPAIRS * COUT], bf16)
        nc.sync.dma_start(wt[:], wts[:])
        bt = const.tile([COUT, 1], f32)
        nc.sync.dma_start(bt[:], biasd[:])

        ip = ctx.enter_context(tc.tile_pool(name="ip", bufs=3))
        gp = ctx.enter_context(tc.tile_pool(name="gp", bufs=6))
        tp = ctx.enter_context(tc.tile_pool(name="tp", bufs=2, space="PSUM"))
        rp = ctx.enter_context(tc.tile_pool(name="rp", bufs=6))
        app = ctx.enter_context(tc.tile_pool(name="ap", bufs=2, space="PSUM"))
        bp = ctx.enter_context(tc.tile_pool(name="bp", bufs=2))
        otp = ctx.enter_context(tc.tile_pool(name="otp", bufs=2, space="PSUM"))
        osp = ctx.enter_context(tc.tile_pool(name="osp", bufs=3))

        gq = [0]

        def gather(out_ap, idx_ap):
            inst = nc.gpsimd.indirect_dma_start(
                out=out_ap,
                out_offset=None,
                in_=tbl[:],
                in_offset=bass.IndirectOffsetOnAxis(ap=idx_ap, axis=0),
            )
            if N_SWDGE_QUEUES > 1:
                q = gq[0] % N_SWDGE_QUEUES
                gq[0] += 1
                if q:
                    inst.ins.queue = f"qPoolDynamic{q}"
            return inst

        for t in range(cfg.n_tiles):
            it = ip.tile([128, IDX_PER_TILE], i32)
            nc.sync.dma_start(it[:], idxs[t])
            acc = app.tile([COUT, TILE], f32)
            for pr in range(NPAIRS):
                # HW indirect DMA: one index per partition per instruction.
                # 8 gathers fill g [128, 512]: chunk c, lane w at cols
                # c*128 + w*64. Index column q = pr*8 + c*2 + w.
                g = gp.tile([128, 512], bf16)
                for c in range(4):
                    for w in range(2):
                        q = pr * 8 + c * 2 + w
                        gather(
                            g[:, c * 128 + w * 64 : c * 128 + (w + 1) * 64],
                            it[:, q : q + 1],
                        )
                tpt = tp.tile([128, TILE], bf16)
                for c in range(4):
                    nc.tensor.transpose(
                        out=tpt[:, c * 128 : (c + 1) * 128],
                        in_=g[:, c * 128 : (c + 1) * 128],
                        identity=idb[:],
                    )
                r = rp.tile([128, TILE], bf16)
                if pr % 2 == 0:
                    nc.vector.tensor_copy(r[:], tpt[:])
                else:
                    nc.scalar.copy(r[:], tpt[:])
                nc.tensor.matmul(
                    acc[:],
                    wt[:, pr * COUT : (pr + 1) * COUT],
                    r[:],
                    start=(pr == 0),
                    stop=(pr == NPAIRS - 1),
                )
            ob = bp.tile([COUT, TILE], f32)
            nc.vector.tensor_add(
                out=ob[:], in0=acc[:], in1=bt[:].to_broadcast([COUT, TILE])
            )
            ot = otp.tile([128, 4 * COUT], f32)
            for c in range(4):
                nc.tensor.transpose(
                    out=ot[:, c * COUT : (c + 1) * COUT],
                    in_=ob[:, c * 128 : (c + 1) * 128],
                    identity=idf[:COUT, :COUT],
                )
            os_ = osp.tile([128, 4 * COUT], f32)
            nc.scalar.copy(os_[:], ot[:])
            nc.sync.dma_start(
                outd[t * TILE : (t + 1) * TILE, :].rearrange(
                    "(c p) ci -> p c ci", p=128
                ),
                os_[:].rearrange("p (c ci) -> p c ci", c=4),
            )

    nc.compile()
    return nc


def prep_inputs(feats, kern, bias, neighbor_map, cfg: Cfg, n_sites_total, n_cores):
    """Host-side marshalling into per-core input maps."""
    zrow = n_sites_total  # index of the zero row in the padded table
    assert cfg.n_rows > zrow

    tblh = np.zeros((cfg.n_rows, CIN), dtype=BF16)
    tblh[: feats.shape[0]] = feats.astype(BF16)

    nm = np.asarray(neighbor_map)
    idx32 = np.where(nm >= 0, nm, zrow).astype(np.int32)  # [KVOL, n_sites]

    w_pk = np.zeros((NPAIRS, 128, COUT), dtype=np.float32)
    for pr in range(NPAIRS):
        k0, k1 = 2 * pr, 2 * pr + 1
        w_pk[pr, :CIN] = kern[k0]
        if k1 < KVOL:
            w_pk[pr, CIN:] = kern[k1]
    wtsh = np.ascontiguousarray(
        w_pk.transpose(1, 0, 2).reshape(128, NPAIRS * COUT)
    ).astype(BF16)

    biash = np.ascontiguousarray(bias.reshape(COUT, 1)).astype(np.float32)

    spc = n_sites_total // n_cores
    in_maps = []
    for c in range(n_cores):
        sl = idx32[:, c * spc : (c + 1) * spc]  # [27, spc]
        padn = cfg.spad - spc
        a = np.concatenate(
            [
                np.concatenate(
                    [sl, np.full((KVOL, padn), zrow, np.int32)], axis=1
                ),
                np.full((1, cfg.spad), zrow, np.int32),
            ],
            axis=0,
        )  # [28, spad]
        a = a.reshape(2 * NPAIRS, cfg.n_tiles, 4, 128)  # [k, t, c, p]
        a = a.reshape(NPAIRS, 2, cfg.n_tiles, 4, 128).transpose(2, 4, 0, 3, 1)
        idxh = np.ascontiguousarray(
            a.reshape(cfg.n_tiles, 128, IDX_PER_TILE)
        )
        in_maps.append({"tbl": tblh, "idxs": idxh, "wts": wtsh, "bias": biash})
    return in_maps


_CACHE = {}


def kernel(feats, kernel, bias, neighbor_map):
    feats = np.asarray(feats, dtype=np.float32)
    kern = np.asarray(kernel, dtype=np.float32)
    bias = np.asarray(bias, dtype=np.float32)

    n_tiles = (SPC + TILE - 1) // TILE  # 98
    cfg = Cfg(n_rows=N + 128, n_tiles=n_tiles)

    if "nc" not in _CACHE:
        _CACHE["nc"] = build(cfg)
    nc = _CACHE["nc"]

    in_maps = prep_inputs(feats, kern, bias, neighbor_map, cfg, N, NCORES)
    res = run_bass_kernel_spmd(nc, in_maps, list(range(NCORES)))
    out = np.concatenate(
        [res.results[i]["out"][:SPC] for i in range(NCORES)], axis=0
    )
    return out.astype(np.float32)


if __name__ == "__main__":
    # smoke test with random data
    rng = np.random.default_rng(0)
    feats = rng.standard_normal((N, CIN), dtype=np.float32)
    kern = rng.standard_normal((KVOL, CIN, COUT), dtype=np.float32) * 0.02
    bias = rng.standard_normal(COUT).astype(np.float32) * 0.02
    nm = rng.integers(0, N, (KVOL, N))
    out = kernel(feats, kern, bias, nm)
    print(out.shape, out.dtype)



# revision 5
# speedup vs baseline: 1.9168x; 1.3663x over previous
"""Sparse 3D conv (rulebook gather -> GEMM -> accumulate) on 8 TRN2 NeuronCores.

Strategy (data-parallel over output sites, no collectives):
  - Replicate the feats table (bf16) and the [27,64,64] kernel on every core.
  - Shard the 400k output sites 50k/core; each core gathers its neighbor
    rows via SWDGE indirect DMA. Invalid rulebook entries (~40%) are
    marked with an out-of-bounds index and skipped by the DMA's bounds
    check (gather tiles are zero-filled first), cutting SWDGE descriptor
    generation - the kernel's bottleneck - by ~40%.
  - The always-valid center offset (k=13) reads contiguous rows, so it is
    paired with the zero-pad lane and served by a direct HWDGE load from a
    per-core contiguous copy of its feats slice instead of indirect DMA.
  - Gathered [site, cin] tiles are transposed to [cin, site] on the
    TensorEngine (pairs of offsets packed to K=128), and 14 K=128-packed
    bf16 matmuls accumulate into f32 PSUM. Bias add fused into the
    PSUM->SBUF copy, output transposed back to site-major and DMAd out.
"""

import sys
from contextlib import ExitStack

sys.path.insert(0, "/opt/trn_rl_repo")

import ml_dtypes
import numpy as np

import concourse.bass as bass
import concourse.tile as tile
from concourse import bacc, mybir
from concourse.bass_utils import run_bass_kernel_spmd
from concourse.masks import make_identity

BF16 = ml_dtypes.bfloat16

# Problem constants (hardcoded per contract)
N = 400000
CIN = 64
COUT = 64
KVOL = 27
NCORES = 8
SPC = N // NCORES  # sites per core = 50000

TILE = 512  # sites per device tile
NPAIRS = 14  # 13 random-offset pairs + (center, pad)
NRAND = 13  # pairs served by indirect gather
IDX_PER_TILE = NRAND * 8  # 104 indices per partition per tile
OOB = 1 << 29  # index marker for invalid entries (> bounds check)

# offsets 0-12,14-26 feed the 13 random pairs; 13 (center) pairs with pad
PERM26 = list(range(13)) + list(range(14, 27))


class Cfg:
    def __init__(self, n_rows, n_tiles):
        self.n_rows = n_rows  # feats table rows
        self.n_tiles = n_tiles  # site tiles per core
        self.spad = n_tiles * TILE  # padded sites per core


N_SWDGE_QUEUES = 4  # spread indirect gathers across SWDGE queues


def build(cfg: Cfg):
    """Build + compile the per-core Bass program. Returns nc."""
    nc = bacc.Bacc(
        "TRN2",
        target_bir_lowering=False,
        debug=False,
        num_devices=NCORES,
        num_swdge_queues=N_SWDGE_QUEUES,
    )
    f32 = mybir.dt.float32
    bf16 = mybir.dt.bfloat16
    i32 = mybir.dt.int32

    tbl = nc.dram_tensor("tbl", [cfg.n_rows, CIN], bf16, kind="ExternalInput")
    ctbl = nc.dram_tensor("ctbl", [cfg.spad, CIN], bf16, kind="ExternalInput")
    idxs = nc.dram_tensor(
        "idxs", [cfg.n_tiles, 128, IDX_PER_TILE], i32, kind="ExternalInput"
    )
    wts = nc.dram_tensor("wts", [128, NPAIRS * COUT], bf16, kind="ExternalInput")
    biasd = nc.dram_tensor("bias", [COUT, 1], f32, kind="ExternalInput")
    outd = nc.dram_tensor("out", [cfg.spad, COUT], f32, kind="ExternalOutput")

    with tile.TileContext(nc) as tc, ExitStack() as ctx:
        const = ctx.enter_context(tc.tile_pool(name="const", bufs=1))
        idf = const.tile([128, 128], f32)
        make_identity(nc, idf[:])
        idb = const.tile([128, 128], bf16)
        nc.vector.tensor_copy(idb[:], idf[:])
        wt = const.tile([128, NPAIRS * COUT], bf16)
        nc.sync.dma_start(wt[:], wts[:])
        bt = const.tile([COUT, 1], f32)
        nc.sync.dma_start(bt[:], biasd[:])
        zt = const.tile([128, 512], bf16)
        nc.vector.memset(zt[:], 0)

        ip = ctx.enter_context(tc.tile_pool(name="ip", bufs=3))
        gp = ctx.enter_context(tc.tile_pool(name="gp", bufs=6))
        tp = ctx.enter_context(tc.tile_pool(name="tp", bufs=2, space="PSUM"))
        rp = ctx.enter_context(tc.tile_pool(name="rp", bufs=6))
        app = ctx.enter_context(tc.tile_pool(name="ap", bufs=2, space="PSUM"))
        bp = ctx.enter_context(tc.tile_pool(name="bp", bufs=2))
        otp = ctx.enter_context(tc.tile_pool(name="otp", bufs=2, space="PSUM"))
        osp = ctx.enter_context(tc.tile_pool(name="osp", bufs=3))

        gq = [0]

        def gather(out_ap, idx_ap):
            inst = nc.gpsimd.indirect_dma_start(
                out=out_ap,
                out_offset=None,
                in_=tbl[:],
                in_offset=bass.IndirectOffsetOnAxis(ap=idx_ap, axis=0),
                bounds_check=N,
                oob_is_err=False,
            )
            q = gq[0] % N_SWDGE_QUEUES
            gq[0] += 1
            if q:
                inst.ins.queue = f"qPoolDynamic{q}"
            return inst

        for t in range(cfg.n_tiles):
            it = ip.tile([128, IDX_PER_TILE], i32)
            nc.sync.dma_start(it[:], idxs[t])
            acc = app.tile([COUT, TILE], f32)
            for pr in range(NPAIRS):
                # HW indirect DMA: one index per partition per instruction.
                # 8 gathers fill g [128, 512]: chunk c, lane w at cols
                # c*128 + w*64. Index column q = pr*8 + c*2 + w.
                g = gp.tile([128, 512], bf16)
                if pr < NRAND:
                    # skipped OOB entries leave the zero fill in place
                    if pr % 2 == 0:
                        nc.vector.memset(g[:], 0)
                    else:
                        nc.scalar.copy(g[:], zt[:])
                    for c in range(4):
                        for w in range(2):
                            q = pr * 8 + c * 2 + w
                            gather(
                                g[:, c * 128 + w * 64 : c * 128 + (w + 1) * 64],
                                it[:, q : q + 1],
                            )
                else:
                    # (center, pad) pair: lane 0 is a contiguous direct
                    # load of this tile's own sites, lane 1 stays zero.
                    nc.scalar.copy(g[:], zt[:])
                    for c in range(4):
                        r0 = t * TILE + c * 128
                        nc.sync.dma_start(
                            g[:, c * 128 : c * 128 + CIN],
                            ctbl[r0 : r0 + 128, :],
                        )
                tpt = tp.tile([128, TILE], bf16)
                for c in range(4):
                    nc.tensor.transpose(
                        out=tpt[:, c * 128 : (c + 1) * 128],
                        in_=g[:, c * 128 : (c + 1) * 128],
                        identity=idb[:],
                    )
                r = rp.tile([128, TILE], bf16)
                if pr % 2 == 0:
                    nc.vector.tensor_copy(r[:], tpt[:])
                else:
                    nc.scalar.copy(r[:], tpt[:])
                nc.tensor.matmul(
                    acc[:],
                    wt[:, pr * COUT : (pr + 1) * COUT],
                    r[:],
                    start=(pr == 0),
                    stop=(pr == NPAIRS - 1),
                )
            ob = bp.tile([COUT, TILE], f32)
            nc.vector.tensor_add(
                out=ob[:], in0=acc[:], in1=bt[:].to_broadcast([COUT, TILE])
            )
            ot = otp.tile([128, 4 * COUT], f32)
            for c in range(4):
                nc.tensor.transpose(
                    out=ot[:, c * COUT : (c + 1) * COUT],
                    in_=ob[:, c * 128 : (c + 1) * 128],
                    identity=idf[:COUT, :COUT],
                )
            os_ = osp.tile([128, 4 * COUT], f32)
            nc.scalar.copy(os_[:], ot[:])
            nc.sync.dma_start(
                outd[t * TILE : (t + 1) * TILE, :].rearrange(
                    "(c p) ci -> p c ci", p=128
                ),
                os_[:].rearrange("p (c ci) -> p c ci", c=4),
            )

    nc.compile()
    return nc


def prep_inputs(feats, kern, bias, neighbor_map, cfg: Cfg, n_sites_total, n_cores):
    """Host-side marshalling into per-core input maps."""
    tblh = np.zeros((cfg.n_rows, CIN), dtype=BF16)
    tblh[: feats.shape[0]] = feats.astype(BF16)

    nm = np.asarray(neighbor_map)
    idx32 = np.where(nm >= 0, nm, OOB).astype(np.int32)  # [KVOL, n_sites]

    # weights: 13 random pairs in PERM26 order, then (center, zero-pad)
    w_pk = np.zeros((NPAIRS, 128, COUT), dtype=np.float32)
    for pr in range(NRAND):
        k0, k1 = PERM26[2 * pr], PERM26[2 * pr + 1]
        w_pk[pr, :CIN] = kern[k0]
        w_pk[pr, CIN:] = kern[k1]
    w_pk[NRAND, :CIN] = kern[13]
    wtsh = np.ascontiguousarray(
        w_pk.transpose(1, 0, 2).reshape(128, NPAIRS * COUT)
    ).astype(BF16)

    biash = np.ascontiguousarray(bias.reshape(COUT, 1)).astype(np.float32)

    spc = n_sites_total // n_cores
    in_maps = []
    for c in range(n_cores):
        sl = idx32[PERM26, c * spc : (c + 1) * spc]  # [26, spc]
        padn = cfg.spad - spc
        a = np.concatenate([sl, np.full((26, padn), OOB, np.int32)], axis=1)
        a = a.reshape(26, cfg.n_tiles, 4, 128)  # [k, t, c, p]
        a = a.reshape(NRAND, 2, cfg.n_tiles, 4, 128).transpose(2, 4, 0, 3, 1)
        idxh = np.ascontiguousarray(a.reshape(cfg.n_tiles, 128, IDX_PER_TILE))

        ctblh = np.zeros((cfg.spad, CIN), dtype=BF16)
        ctblh[:spc] = feats[c * spc : (c + 1) * spc].astype(BF16)
        in_maps.append(
            {"tbl": tblh, "ctbl": ctblh, "idxs": idxh, "wts": wtsh, "bias": biash}
        )
    return in_maps


_CACHE = {}


def kernel(feats, kernel, bias, neighbor_map):
    feats = np.asarray(feats, dtype=np.float32)
    kern = np.asarray(kernel, dtype=np.float32)
    bias = np.asarray(bias, dtype=np.float32)

    n_tiles = (SPC + TILE - 1) // TILE  # 98
    cfg = Cfg(n_rows=N + 128, n_tiles=n_tiles)

    if "nc" not in _CACHE:
        _CACHE["nc"] = build(cfg)
    nc = _CACHE["nc"]

    in_maps = prep_inputs(feats, kern, bias, neighbor_map, cfg, N, NCORES)
    res = run_bass_kernel_spmd(nc, in_maps, list(range(NCORES)))
    out = np.concatenate(
        [res.results[i]["out"][:SPC] for i in range(NCORES)], axis=0
    )
    return out.astype(np.float32)


if __name__ == "__main__":
    # smoke test with random data
    rng = np.random.default_rng(0)
    feats = rng.standard_normal((N, CIN), dtype=np.float32)
    kern = rng.standard_normal((KVOL, CIN, COUT), dtype=np.float32) * 0.02
    bias = rng.standard_normal(COUT).astype(np.float32) * 0.02
    nm = rng.integers(0, N, (KVOL, N))
    out = kernel(feats, kern, bias, nm)
    print(out.shape, out.dtype)
